# revision 4
# baseline (speedup 1.0000x reference)
"""ConditionalVAE (soft-MoE decoder) Trainium2 kernel.

Data-parallel over 8 NeuronCores: batch 8192 -> 1024 per core, all weights
replicated. Encoder runs with activations in [feature, batch] layout
("transposed"), so layer outputs chain directly into the next layer's moving
operand. Decoder runs with batch on partitions so the per-sample expert
coefficients become per-partition scalars (fused multiply-accumulate via
scalar_tensor_tensor) and the expert bias terms become K=4 seed matmuls.
All heavy matmuls use the fp32r tensor-engine mode (1 cycle/row).
Weights are streamed through SBUF as [128, 512] tiles, each consumed by the
matmuls of one (k-tile, n-half) immediately after arrival.
"""
import sys
sys.path.insert(0, '/opt/trn_rl_repo')

import numpy as np
import concourse.bass as bass
import concourse.tile as tile
from concourse import bacc, mybir
from concourse import bass_utils
from concourse.masks import make_identity

f32 = mybir.dt.float32
f32r = mybir.dt.float32r
AF = mybir.ActivationFunctionType
ALU = mybir.AluOpType

B, IN, COND, HID, LAT, E = 8192, 512, 256, 1024, 128, 4
EPSC = 1e-4
NCORES = 8
BL = B // NCORES          # 1024 batch rows per core
BT = BL // 128            # 8 batch tiles
IN_T, CO_T, HID_T = IN // 128, COND // 128, HID // 128  # 4, 2, 8
CSL = [slice(c * 512, (c + 1) * 512) for c in range(BL // 512)]  # moving chunks

TRACE = False
LAST_EXEC_NS = None
_BUILT = None


def _build():
    nc = bacc.Bacc("TRN2", target_bir_lowering=False, debug=False,
                   enable_asserts=False, num_devices=NCORES)
    d = {}

    def din(name, shape, dt):
        d[name] = nc.dram_tensor(name, list(shape), dt, kind="ExternalInput").ap()

    def dout(name, shape, dt=f32):
        d[name] = nc.dram_tensor(name, list(shape), dt, kind="ExternalOutput").ap()

    din('x', (BL, IN), f32)
    din('w', (BL, COND), f32)
    din('eps', (BL, LAT), f32)
    din('in_s', (IN,), f32); din('in_nb', (IN,), f32)
    din('c_s', (COND,), f32); din('c_nb', (COND,), f32)
    din('ew0', (IN + COND, HID), f32r)
    for L in (1, 2, 3):
        din(f'ew{L}', (COND + HID, HID), f32r)
    for L in range(4):
        din(f'eb{L}', (HID,), f32)
    din('mu_w', (HID, LAT), f32r); din('mu_b', (LAT,), f32r)
    din('lv_w', (HID, LAT), f32r); din('lv_b', (LAT,), f32r)
    din('gw0', (LAT + COND, 64), f32r); din('gb0', (64,), f32)
    din('gw1', (64, 64), f32r); din('gb1', (64,), f32)
    din('gw2', (64, E), f32r); din('gb2', (E,), f32)
    din('dw0', (E, LAT + COND, HID), f32r); din('db0', (E, HID), f32r)
    for L in (1, 2, 3):
        din(f'dw{L}', (E, LAT + COND + HID, HID), f32r)
        din(f'db{L}', (E, HID), f32r)
    din('dw4', (E, LAT + COND + HID, IN), f32r); din('db4', (E, IN), f32r)
    din('ones', (128, 128), f32r)

    dout('z_o', (BL, LAT))
    dout('out_o', (BL, IN))
    dout('mu_o', (BL, LAT))
    dout('lv_o', (BL, LAT))

    with tile.TileContext(nc) as tc:
        with tc.tile_pool(name="const", bufs=1) as cp, \
             tc.tile_pool(name="hbuf", bufs=2) as hp, \
             tc.tile_pool(name="wt", bufs=16) as wp, \
             tc.tile_pool(name="ps", bufs=8, space="PSUM") as pp:

            def ps_tile(nm):
                return pp.tile([128, 512], f32, name=nm, tag="ps")

            def wtile(nm):
                return wp.tile([128, 512], f32r, name=nm, tag="wt")

            ident = cp.tile([128, 128], f32)
            make_identity(nc, ident[:])
            ones_r = cp.tile([128, 128], f32r)
            nc.sync.dma_start(ones_r[:], d['ones'][:, :])

            # per-partition norm scale/bias columns
            in_s_sb = cp.tile([128, IN_T], f32)
            in_nb_sb = cp.tile([128, IN_T], f32)
            c_s_sb = cp.tile([128, CO_T], f32)
            c_nb_sb = cp.tile([128, CO_T], f32)
            for t in range(IN_T):
                nc.sync.dma_start(in_s_sb[:, t:t+1], d['in_s'][t*128:(t+1)*128].unsqueeze(1))
                nc.sync.dma_start(in_nb_sb[:, t:t+1], d['in_nb'][t*128:(t+1)*128].unsqueeze(1))
            for t in range(CO_T):
                nc.sync.dma_start(c_s_sb[:, t:t+1], d['c_s'][t*128:(t+1)*128].unsqueeze(1))
                nc.sync.dma_start(c_nb_sb[:, t:t+1], d['c_nb'][t*128:(t+1)*128].unsqueeze(1))

            eb_sb = cp.tile([128, 4, HID_T], f32)
            for L in range(4):
                for n in range(HID_T):
                    nc.sync.dma_start(eb_sb[:, L, n:n+1], d[f'eb{L}'][n*128:(n+1)*128].unsqueeze(1))

            wn_T = cp.tile([128, CO_T, BL], f32r)
            z_T = cp.tile([128, BL], f32r)
            coeff_sb = cp.tile([4, BL], f32r)
            coeff_nat = cp.tile([128, BT, E], f32)
            db_sb = {}
            for L in range(5):
                n_out = IN if L == 4 else HID
                t_ = cp.tile([4, n_out], f32r, name=f'db{L}_sb')
                nc.sync.dma_start(t_[:], d[f'db{L}'][:, :])
                db_sb[L] = t_

            # ---------------- phase A: input transpose + normalize ------------
            with tc.tile_pool(name="xp", bufs=1) as xp, \
                 tc.tile_pool(name="xnat", bufs=3) as xnp:
                xn_T = xp.tile([128, IN_T, BL], f32r)
                for bt in range(BT):
                    bsl = slice(bt*128, (bt+1)*128)
                    x_nat = xnp.tile([128, 512], f32, name="x_nat", tag="xnat")
                    nc.sync.dma_start(x_nat[:, :IN], d['x'][bsl, :])
                    xa = xnp.tile([128, IN_T, 128], f32, name="xa", tag="xa")
                    for ft in range(IN_T):
                        pst = ps_tile("ps_tx")
                        nc.tensor.transpose(pst[:, 0:128], x_nat[:, ft*128:(ft+1)*128], ident[:])
                        nc.scalar.activation(xa[:, ft, :], pst[:, 0:128], AF.Identity,
                                             bias=in_nb_sb[:, ft:ft+1], scale=in_s_sb[:, ft:ft+1])
                    nc.vector.tensor_scalar(xn_T[:, :, bsl], xa[:], -5.0, 5.0, ALU.max, ALU.min)

                    w_nat = xnp.tile([128, 512], f32, name="w_nat", tag="xnat")
                    nc.sync.dma_start(w_nat[:, :COND], d['w'][bsl, :])
                    wa = xnp.tile([128, CO_T, 128], f32, name="wa", tag="wa")
                    for ft in range(CO_T):
                        pst = ps_tile("ps_tw")
                        nc.tensor.transpose(pst[:, 0:128], w_nat[:, ft*128:(ft+1)*128], ident[:])
                        nc.scalar.activation(wa[:, ft, :], pst[:, 0:128], AF.Identity,
                                             bias=c_nb_sb[:, ft:ft+1], scale=c_s_sb[:, ft:ft+1])
                    nc.vector.tensor_scalar(wn_T[:, :, bsl], wa[:], -5.0, 5.0, ALU.max, ALU.min)

                # ------------- encoder (k-streamed weight tiles) --------------
                def enc_layer(L, sources, func):
                    nk = len(sources)
                    h_new = hp.tile([128, HID_T, BL], f32r, name=f"h{L}", tag="h")
                    for ph in range(2):               # n-halves 0-3 / 4-7
                        pss = {}
                        for nn in range(4):
                            for c in range(len(CSL)):
                                pss[(nn, c)] = ps_tile(f"ps_e{L}")
                        for k in range(nk):
                            wt = wtile(f"ew{L}_{ph}_{k}")
                            nc.sync.dma_start(wt[:], d[f'ew{L}'][k*128:(k+1)*128,
                                                                 ph*512:(ph+1)*512])
                            for nn in range(4):
                                for c, csl in enumerate(CSL):
                                    nc.tensor.matmul(pss[(nn, c)][:, 0:512],
                                                     wt[:, nn*128:(nn+1)*128],
                                                     sources[k][:, csl],
                                                     start=(k == 0), stop=(k == nk - 1))
                        for nn in range(4):
                            n = ph * 4 + nn
                            for c, csl in enumerate(CSL):
                                nc.scalar.activation(h_new[:, n, csl], pss[(nn, c)][:, 0:512],
                                                     func, bias=eb_sb[:, L, n:n+1],
                                                     scale=1.0, alpha=0.01)
                    return h_new

                srcs0 = [xn_T[:, k, :] for k in range(IN_T)] + \
                        [wn_T[:, j, :] for j in range(CO_T)]
                h = enc_layer(0, srcs0, AF.Identity)

            for L in (1, 2, 3):
                srcs = [wn_T[:, j, :] for j in range(CO_T)] + \
                       [h[:, k, :] for k in range(HID_T)]
                h = enc_layer(L, srcs, AF.Lrelu if L == 3 else AF.Identity)
            hL = h  # leaky(h3), [128, 8, BL] f32r

            # ---------------- mu / logvar / z (natural layout) ----------------
            with tc.tile_pool(name="zp", bufs=1) as zp:
                mu_w_sb = zp.tile([128, HID_T, LAT], f32r)
                lv_w_sb = zp.tile([128, HID_T, LAT], f32r)
                for k in range(HID_T):
                    nc.sync.dma_start(mu_w_sb[:, k, :], d['mu_w'][k*128:(k+1)*128, :])
                    nc.sync.dma_start(lv_w_sb[:, k, :], d['lv_w'][k*128:(k+1)*128, :])
                mu_b_row = zp.tile([1, LAT], f32r)
                lv_b_row = zp.tile([1, LAT], f32r)
                nc.sync.dma_start(mu_b_row[:], d['mu_b'].unsqueeze(0))
                nc.sync.dma_start(lv_b_row[:], d['lv_b'].unsqueeze(0))
                eps_sb = zp.tile([128, BT, LAT], f32)
                for bt in range(BT):
                    nc.sync.dma_start(eps_sb[:, bt, :], d['eps'][bt*128:(bt+1)*128, :])

                mu_nat = zp.tile([128, BT, LAT], f32)
                lv_pre = zp.tile([128, BT, LAT], f32)
                for bt in range(BT):
                    bsl = slice(bt*128, (bt+1)*128)
                    for w_sb, b_row, dst in ((mu_w_sb, mu_b_row, mu_nat), (lv_w_sb, lv_b_row, lv_pre)):
                        ps = ps_tile("ps_mu")
                        nc.tensor.matmul(ps[:, 0:LAT], ones_r[0:1, 0:128], b_row[0:1, :],
                                         start=True, stop=False)
                        for k in range(HID_T):
                            nc.tensor.matmul(ps[:, 0:LAT], hL[:, k, bsl], w_sb[:, k, :],
                                             start=False, stop=(k == HID_T - 1))
                        nc.scalar.copy(dst[:, bt, :], ps[:, 0:LAT])

                lv_nat = zp.tile([128, BT, LAT], f32)
                nc.vector.tensor_scalar(lv_nat[:], lv_pre[:], -5.0, 5.0, ALU.max, ALU.min)
                e_nat = zp.tile([128, BT, LAT], f32)
                nc.scalar.activation(e_nat[:], lv_nat[:], AF.Exp, bias=0.0, scale=0.5)
                z_nat = zp.tile([128, BT, LAT], f32)
                nc.vector.tensor_tensor(z_nat[:], e_nat[:], eps_sb[:], ALU.mult)
                nc.vector.tensor_tensor(z_nat[:], z_nat[:], mu_nat[:], ALU.add)
                for bt in range(BT):
                    bsl = slice(bt*128, (bt+1)*128)
                    nc.sync.dma_start(d['mu_o'][bsl, :], mu_nat[:, bt, :])
                    nc.sync.dma_start(d['lv_o'][bsl, :], lv_nat[:, bt, :])
                    nc.sync.dma_start(d['z_o'][bsl, :], z_nat[:, bt, :])
                    pst = ps_tile("ps_tz")
                    nc.tensor.transpose(pst[:, 0:128], z_nat[:, bt, :], ident[:])
                    nc.scalar.copy(z_T[:, bsl], pst[:, 0:128])

                # ---------------- gate + softmax coefficients -----------------
                gw0_sb = zp.tile([128, 1 + CO_T, 64], f32r)
                for k in range(1 + CO_T):
                    nc.sync.dma_start(gw0_sb[:, k, :], d['gw0'][k*128:(k+1)*128, :])
                gw1_sb = zp.tile([64, 64], f32r)
                nc.sync.dma_start(gw1_sb[:], d['gw1'][:, :])
                gw2_sb = zp.tile([64, E], f32r)
                nc.sync.dma_start(gw2_sb[:], d['gw2'][:, :])
                gb0_sb = zp.tile([64, 1], f32)
                nc.sync.dma_start(gb0_sb[:], d['gb0'].unsqueeze(1))
                gb1_sb = zp.tile([64, 1], f32)
                nc.sync.dma_start(gb1_sb[:], d['gb1'].unsqueeze(1))
                gb2_sb = zp.tile([4, 1], f32)
                nc.sync.dma_start(gb2_sb[:], d['gb2'].unsqueeze(1))

                g0T = zp.tile([64, BL], f32r)
                g1T = zp.tile([64, BL], f32r)
                expt = zp.tile([4, BL], f32r)
                r_f = zp.tile([1, BL], f32)
                r_r = zp.tile([1, BL], f32r)
                coeff_f32 = zp.tile([4, BL], f32)
                for c, csl in enumerate(CSL):
                    ps = ps_tile("ps_g0")
                    nc.tensor.matmul(ps[0:64, 0:512], gw0_sb[:, 0, :], z_T[:, csl],
                                     start=True, stop=False)
                    for j in range(CO_T):
                        nc.tensor.matmul(ps[0:64, 0:512], gw0_sb[:, 1+j, :], wn_T[:, j, csl],
                                         start=False, stop=(j == CO_T - 1))
                    nc.scalar.activation(g0T[:, csl], ps[0:64, 0:512], AF.Lrelu,
                                         bias=gb0_sb[:, 0:1], scale=1.0, alpha=0.01)
                    ps1 = ps_tile("ps_g1")
                    nc.tensor.matmul(ps1[0:64, 0:512], gw1_sb[0:64, :], g0T[0:64, csl],
                                     start=True, stop=True)
                    nc.scalar.activation(g1T[:, csl], ps1[0:64, 0:512], AF.Lrelu,
                                         bias=gb1_sb[:, 0:1], scale=1.0, alpha=0.01)
                    ps2 = ps_tile("ps_g2")
                    nc.tensor.matmul(ps2[0:4, 0:512], gw2_sb[0:64, 0:E], g1T[0:64, csl],
                                     start=True, stop=True)
                    # exp(logits + gb2), no max-subtraction (logits are tiny)
                    nc.scalar.activation(expt[:, csl], ps2[0:4, 0:512], AF.Exp,
                                         bias=gb2_sb[:, 0:1], scale=1.0)
                    ps3 = ps_tile("ps_gs")
                    nc.tensor.matmul(ps3[0:1, 0:512], ones_r[0:4, 0:1], expt[0:4, csl],
                                     start=True, stop=True)
                    nc.vector.reciprocal(r_f[0:1, csl], ps3[0:1, 0:512])
                    nc.vector.tensor_copy(r_r[0:1, csl], r_f[0:1, csl])
                    ps4 = ps_tile("ps_gb")
                    nc.tensor.matmul(ps4[0:4, 0:512], ones_r[0:1, 0:4], r_r[0:1, csl],
                                     start=True, stop=True)
                    nc.vector.tensor_tensor(coeff_sb[0:4, csl], expt[0:4, csl],
                                            ps4[0:4, 0:512], ALU.mult)
                nc.vector.tensor_copy(coeff_f32[:], coeff_sb[:])
                for bt in range(BT):
                    pst = ps_tile("ps_tc")
                    nc.tensor.transpose(pst[:, 0:4], coeff_f32[0:4, bt*128:(bt+1)*128],
                                        ident[0:4, 0:4])
                    nc.scalar.copy(coeff_nat[:, bt, :], pst[:, 0:4])

            # ---------------- decoder -----------------------------------------
            with tc.tile_pool(name="accp", bufs=8) as ap_:

                def dec_layer(L, n_out, sources, out_T):
                    nk = len(sources)
                    nch = n_out // 512
                    dwd = d[f'dw{L}']
                    accs = []
                    for bt in range(BT):
                        bsl = slice(bt*128, (bt+1)*128)
                        acc = ap_.tile([128, 1024], f32, name=f"acc{L}_{bt}", tag="acc")
                        accs.append(acc)
                        for c in range(nch):
                            ps = ps_tile(f"ps_sd{L}")
                            nc.tensor.matmul(ps[:, 0:512], coeff_sb[0:4, bsl],
                                             db_sb[L][0:4, c*512:(c+1)*512],
                                             start=True, stop=True)
                            nc.scalar.copy(acc[:, c*512:(c+1)*512], ps[:, 0:512])
                    for c in range(nch):
                        osl = slice(c*512, (c+1)*512)
                        for e in range(E):
                            pss = [ps_tile(f"ps_d{L}") for _ in range(BT)]
                            for k in range(nk):
                                wt = wtile(f"dw{L}_{c}_{e}_{k}")
                                nc.sync.dma_start(wt[:], dwd[e, k*128:(k+1)*128, osl])
                                for bt in range(BT):
                                    nc.tensor.matmul(pss[bt][:, 0:512], sources[k](bt),
                                                     wt[:, 0:512],
                                                     start=(k == 0), stop=(k == nk - 1))
                            for bt in range(BT):
                                nc.vector.scalar_tensor_tensor(
                                    accs[bt][:, osl], pss[bt][:, 0:512],
                                    coeff_nat[:, bt, e:e+1], accs[bt][:, osl],
                                    ALU.mult, ALU.add)
                    if out_T is not None:
                        for bt in range(BT):
                            bsl = slice(bt*128, (bt+1)*128)
                            for nt in range(n_out // 128):
                                pst = ps_tile(f"ps_td{L}")
                                nc.tensor.transpose(pst[:, 0:128],
                                                    accs[bt][:, nt*128:(nt+1)*128], ident[:])
                                nc.scalar.activation(out_T[:, nt, bsl], pst[:, 0:128],
                                                     AF.Lrelu, bias=0.0, scale=1.0,
                                                     alpha=0.01)
                    else:
                        for bt in range(BT):
                            nc.sync.dma_start(d['out_o'][bt*128:(bt+1)*128, :],
                                              accs[bt][:, 0:n_out])

                def src_z(bt):
                    return z_T[:, bt*128:(bt+1)*128]

                def src_wn(j):
                    return lambda bt: wn_T[:, j, bt*128:(bt+1)*128]

                def src_h(hbuf, k):
                    return lambda bt: hbuf[:, k, bt*128:(bt+1)*128]

                zc_sources = [src_z] + [src_wn(j) for j in range(CO_T)]

                out0_T = hp.tile([128, HID_T, BL], f32r, name="out0T", tag="h")
                dec_layer(0, HID, zc_sources, out0_T)
                prev = out0_T
                for L in (1, 2, 3):
                    outL_T = hp.tile([128, HID_T, BL], f32r, name=f"out{L}T", tag="h")
                    dec_layer(L, HID, zc_sources + [src_h(prev, k) for k in range(HID_T)],
                              outL_T)
                    prev = outL_T
                dec_layer(4, IN, zc_sources + [src_h(prev, k) for k in range(HID_T)],
                          None)

    nc.compile()
    return nc


def _get_nc():
    global _BUILT
    if _BUILT is None:
        _BUILT = _build()
    return _BUILT


def kernel(**inputs):
    global LAST_EXEC_NS
    nc = _get_nc()
    f = lambda a: np.ascontiguousarray(np.asarray(a, dtype=np.float32))

    in_s = 1.0 / np.sqrt(f(inputs['rms_in_var']) + EPSC)
    in_nb = -f(inputs['rms_in_mean']) * in_s
    c_s = 1.0 / np.sqrt(f(inputs['rms_c_var']) + EPSC)
    c_nb = -f(inputs['rms_c_mean']) * c_s

    shared = dict(
        in_s=in_s, in_nb=in_nb, c_s=c_s, c_nb=c_nb,
        ew0=f(inputs['enc_w0']), ew1=f(inputs['enc_w1']),
        ew2=f(inputs['enc_w2']), ew3=f(inputs['enc_w3']),
        eb0=f(inputs['enc_b0']), eb1=f(inputs['enc_b1']),
        eb2=f(inputs['enc_b2']), eb3=f(inputs['enc_b3']),
        mu_w=f(inputs['mu_w']), mu_b=f(inputs['mu_b']),
        lv_w=f(inputs['lv_w']), lv_b=f(inputs['lv_b']),
        gw0=f(inputs['g_w0']), gb0=f(inputs['g_b0']),
        gw1=f(inputs['g_w1']), gb1=f(inputs['g_b1']),
        gw2=f(inputs['g_w2']), gb2=f(inputs['g_b2']),
        dw0=f(inputs['dec_w0']), db0=f(inputs['dec_b0']),
        dw1=f(inputs['dec_w1']), db1=f(inputs['dec_b1']),
        dw2=f(inputs['dec_w2']), db2=f(inputs['dec_b2']),
        dw3=f(inputs['dec_w3']), db3=f(inputs['dec_b3']),
        dw4=f(inputs['dec_w4']), db4=f(inputs['dec_b4']),
        ones=np.ones((128, 128), np.float32),
    )
    x, w, eps = f(inputs['x']), f(inputs['w']), f(inputs['eps'])
    in_maps = []
    for c in range(NCORES):
        sl = slice(c * BL, (c + 1) * BL)
        m = dict(shared)
        m['x'], m['w'], m['eps'] = x[sl], w[sl], eps[sl]
        in_maps.append(m)

    res = bass_utils.run_bass_kernel_spmd(nc, in_maps, core_ids=list(range(NCORES)),
                                          trace=TRACE)
    LAST_EXEC_NS = res.exec_time_ns
    z = np.concatenate([res.results[c]['z_o'] for c in range(NCORES)], axis=0)
    out = np.concatenate([res.results[c]['out_o'] for c in range(NCORES)], axis=0)
    mu = np.concatenate([res.results[c]['mu_o'] for c in range(NCORES)], axis=0)
    lv = np.concatenate([res.results[c]['lv_o'] for c in range(NCORES)], axis=0)
    return z, out, mu, lv
